# revision 1
# baseline (speedup 1.0000x reference)
"""Conv4d (B=2, Ci=32, Co=64, 16^4 spatial, k=3^4, stride 1, pad 1) on 8
Trainium2 NeuronCores.

Sharding: 8 cores = batch(2) x T-quarters(4). Each core computes
out[64co, 4t, 16d, 16h, 16w] for its (b, t-quarter).

Per-core layout: SBUF x tile [128, 6t*6d*324] where partition group
r in {0..3} holds ci=32 channels of the padded input restricted to the
D-halo window [4r, 4r+6) (plus T halo), planes flattened as 18x18=324.
The 4 partition groups process the 4 output-D-quarters concurrently via
PE row-group tiling (tile_position=(32r, 0)).

Each output (t, d-pair) plane-pair accumulates 81 tap matmuls
(K=32ci, M=64co, N=512=2d*16h*16w) in fp32r (TF32) into one PSUM bank
per row group; epilogue adds bias (DVE/ACT) and DMAs out.
"""
import sys

sys.path.insert(0, "/opt/trn_rl_repo")
import numpy as np

N_CORES = 8
TAPS = [(kt, kd, kh, kw) for kt in range(3) for kd in range(3)
        for kh in range(3) for kw in range(3)]

_NC = None


def _build():
    global _NC
    if _NC is not None:
        return _NC
    import concourse.bacc as bacc
    import concourse.tile as tile
    from concourse import mybir

    f32 = mybir.dt.float32
    f32r = mybir.dt.float32r

    nc = bacc.Bacc("TRN2", debug=False, target_bir_lowering=False,
                   num_devices=N_CORES)
    xq = nc.dram_tensor("xq", [128, 6 * 6 * 324], f32r, kind="ExternalInput")
    wq = nc.dram_tensor("wq", [32, 81 * 64], f32r, kind="ExternalInput")
    bq = nc.dram_tensor("biasq", [64, 1], f32, kind="ExternalInput")
    out = nc.dram_tensor("out", [64, 16384], f32, kind="ExternalOutput")

    with tile.TileContext(nc) as tc:
        with tc.tile_pool(name="xp", bufs=1) as xp, \
             tc.tile_pool(name="wp", bufs=1) as wp, \
             tc.tile_pool(name="op", bufs=6) as op_, \
             tc.tile_pool(name="pp", bufs=8, space="PSUM") as pp:
            xtile = xp.tile([128, 11664], f32r)
            for tf in range(6):
                nc.gpsimd.dma_start(xtile[:, tf * 1944:(tf + 1) * 1944],
                                    xq.ap()[:, tf * 1944:(tf + 1) * 1944])
            # weights replicated into all 4 partition groups straight from
            # the small [32, 5184] DRAM copy (4x 0.66MB reads)
            wtile = wp.tile([128, 5184], f32r)
            for r in range(4):
                nc.gpsimd.dma_start(wtile[32 * r:32 * r + 32, :], wq.ap()[:])
            btile = wp.tile([64, 1], f32)
            nc.gpsimd.dma_start(btile[:], bq.ap()[:])

            xv = xtile.rearrange("p (t d h w) -> p t d h w",
                                 t=6, d=6, h=18, w=18)

            for to in range(4):
                for dp in range(2):
                    ps = [pp.tile([64, 512], f32, tag="ps",
                                  name=f"ps_{to}_{dp}_{r}") for r in range(4)]
                    for i, (kt, kd, kh, kw) in enumerate(TAPS):
                        for r in range(4):
                            rhs = xv[32 * r:32 * r + 32, to + kt,
                                     2 * dp + kd: 2 * dp + kd + 2,
                                     kh:kh + 16, kw:kw + 16]
                            lhsT = wtile[32 * r:32 * r + 32,
                                         i * 64:(i + 1) * 64]
                            nc.tensor.matmul(ps[r][:, :], lhsT, rhs,
                                             start=(i == 0), stop=(i == 80),
                                             tile_position=(32 * r, 0))
                    for r in range(4):
                        o = op_.tile([64, 512], f32, tag="ob",
                                     name=f"o_{to}_{dp}_{r}")
                        if r < 2:
                            nc.vector.tensor_scalar_add(o[:], ps[r][:, :],
                                                        btile[:, 0:1])
                        else:
                            nc.scalar.activation(
                                o[:], ps[r][:, :],
                                mybir.ActivationFunctionType.Identity,
                                bias=btile[:, 0:1])
                        off = to * 4096 + (4 * r + 2 * dp) * 256
                        nc.gpsimd.dma_start(out.ap()[:, off:off + 512], o[:])
    nc.compile()
    _NC = nc
    return nc


def _round_tf32(a):
    b = np.ascontiguousarray(a).view(np.uint32)
    r = (b + np.uint32(0x00000FFF) + ((b >> np.uint32(13)) & np.uint32(1))) \
        & np.uint32(0xFFFFE000)
    return r.view(np.float32)


def _prep_inputs(x, weight, bias):
    x = np.asarray(x, dtype=np.float32)
    weight = np.asarray(weight, dtype=np.float32)
    bias = np.asarray(bias, dtype=np.float32)

    w9 = weight.reshape(64, 32, 81).transpose(2, 1, 0)  # [tap, ci, co]
    warr = np.ascontiguousarray(w9.transpose(1, 0, 2)).reshape(32, 81 * 64)
    wq = _round_tf32(warr)
    bq = bias.reshape(64, 1).astype(np.float32)

    in_maps = []
    for b in range(2):
        xpad = np.pad(x[b], ((0, 0), (1, 1), (1, 1), (1, 1), (1, 1)))
        for tq in range(4):
            xt = xpad[:, 4 * tq:4 * tq + 6]  # [32, 6, 18, 18, 18]
            xqc = np.empty((128, 11664), np.float32)
            for r in range(4):
                xqc[32 * r:32 * r + 32] = \
                    xt[:, :, 4 * r:4 * r + 6].reshape(32, -1)
            in_maps.append({"xq": _round_tf32(xqc), "wq": wq, "biasq": bq})
    return in_maps


def run_spmd(x, weight, bias, trace=False, trace_cores=None, tmpdir=None):
    """Returns (output ndarray, BassKernelResults)."""
    from concourse.bass_utils import run_bass_kernel_spmd
    nc = _build()
    in_maps = _prep_inputs(x, weight, bias)
    res = run_bass_kernel_spmd(nc, in_maps, core_ids=list(range(N_CORES)),
                               trace=trace, trace_cores=trace_cores,
                               tmpdir=tmpdir)
    out = np.empty((2, 64, 16, 16, 16, 16), np.float32)
    for c in range(N_CORES):
        b, tq = c // 4, c % 4
        out[b, :, 4 * tq:4 * tq + 4] = \
            res.results[c]["out"].reshape(64, 4, 16, 16, 16)
    return out, res


def kernel(x, weight, bias):
    out, _ = run_spmd(x, weight, bias)
    return out



# revision 2
# speedup vs baseline: 1.2988x; 1.2988x over previous
"""Conv4d (B=2, Ci=32, Co=64, 16^4 spatial, k=3^4, stride 1, pad 1) on 8
Trainium2 NeuronCores.

Sharding: 8 cores = batch(2) x T-quarters(4). Each core computes
out[64co, 4t, 16d, 16h, 16w] for its (b, t-quarter).

Per-core layout: SBUF x tile [128, 6t*6d*324] bf16 where partition group
r in {0..3} holds ci=32 channels of the padded input restricted to the
D-halo window [4r, 4r+6) (plus T halo), planes flattened as 18x18=324.
The 4 partition groups process the 4 output-D-quarters concurrently via
PE row-group tiling (tile_position=(32r, 0)).

Each output (t, d-pair) plane-pair accumulates 81 tap matmuls
(K=32ci, M=64co, N=512=2d*16h*16w) in bf16 into one PSUM bank
per row group; epilogue adds bias (DVE/ACT) and DMAs out.
"""
import sys

sys.path.insert(0, "/opt/trn_rl_repo")
import numpy as np
import ml_dtypes

N_CORES = 8
TAPS = [(kt, kd, kh, kw) for kt in range(3) for kd in range(3)
        for kh in range(3) for kw in range(3)]

_NC = None


def _build():
    global _NC
    if _NC is not None:
        return _NC
    import concourse.bacc as bacc
    import concourse.tile as tile
    from concourse import mybir

    f32 = mybir.dt.float32
    bf16 = mybir.dt.bfloat16

    nc = bacc.Bacc("TRN2", debug=False, target_bir_lowering=False,
                   num_devices=N_CORES)
    xq = nc.dram_tensor("xq", [128, 6 * 6 * 324], bf16, kind="ExternalInput")
    wq = nc.dram_tensor("wq", [32, 81 * 64], bf16, kind="ExternalInput")
    bq = nc.dram_tensor("biasq", [64, 1], f32, kind="ExternalInput")
    out = nc.dram_tensor("out", [64, 16384], f32, kind="ExternalOutput")

    with tile.TileContext(nc) as tc:
        with tc.tile_pool(name="xp", bufs=1) as xp, \
             tc.tile_pool(name="wp", bufs=1) as wp, \
             tc.tile_pool(name="op", bufs=6) as op_, \
             tc.tile_pool(name="pp", bufs=8, space="PSUM") as pp:
            xtile = xp.tile([128, 11664], bf16)
            for tf in range(6):
                nc.gpsimd.dma_start(xtile[:, tf * 1944:(tf + 1) * 1944],
                                    xq.ap()[:, tf * 1944:(tf + 1) * 1944])
            # weights replicated into all 4 partition groups straight from
            # the small [32, 5184] DRAM copy
            wtile = wp.tile([128, 5184], bf16)
            for r in range(4):
                nc.gpsimd.dma_start(wtile[32 * r:32 * r + 32, :], wq.ap()[:])
            btile = wp.tile([64, 1], f32)
            nc.gpsimd.dma_start(btile[:], bq.ap()[:])

            xv = xtile.rearrange("p (t d h w) -> p t d h w",
                                 t=6, d=6, h=18, w=18)

            for to in range(4):
                for dp in range(2):
                    ps = [pp.tile([64, 512], f32, tag="ps",
                                  name=f"ps_{to}_{dp}_{r}") for r in range(4)]
                    for i, (kt, kd, kh, kw) in enumerate(TAPS):
                        for r in range(4):
                            rhs = xv[32 * r:32 * r + 32, to + kt,
                                     2 * dp + kd: 2 * dp + kd + 2,
                                     kh:kh + 16, kw:kw + 16]
                            lhsT = wtile[32 * r:32 * r + 32,
                                         i * 64:(i + 1) * 64]
                            nc.tensor.matmul(ps[r][:, :], lhsT, rhs,
                                             start=(i == 0), stop=(i == 80),
                                             tile_position=(32 * r, 0))
                    for r in range(4):
                        o = op_.tile([64, 512], f32, tag="ob",
                                     name=f"o_{to}_{dp}_{r}")
                        if r < 2:
                            nc.vector.tensor_scalar_add(o[:], ps[r][:, :],
                                                        btile[:, 0:1])
                        else:
                            nc.scalar.activation(
                                o[:], ps[r][:, :],
                                mybir.ActivationFunctionType.Identity,
                                bias=btile[:, 0:1])
                        off = to * 4096 + (4 * r + 2 * dp) * 256
                        nc.gpsimd.dma_start(out.ap()[:, off:off + 512], o[:])
    nc.compile()
    _NC = nc
    return nc


def _prep_inputs(x, weight, bias):
    x = np.asarray(x, dtype=np.float32)
    weight = np.asarray(weight, dtype=np.float32)
    bias = np.asarray(bias, dtype=np.float32)

    w9 = weight.reshape(64, 32, 81).transpose(2, 1, 0)  # [tap, ci, co]
    warr = np.ascontiguousarray(w9.transpose(1, 0, 2)).reshape(32, 81 * 64)
    wq = warr.astype(ml_dtypes.bfloat16)
    bq = bias.reshape(64, 1).astype(np.float32)

    in_maps = []
    for b in range(2):
        xpad = np.pad(x[b], ((0, 0), (1, 1), (1, 1), (1, 1), (1, 1)))
        for tq in range(4):
            xt = xpad[:, 4 * tq:4 * tq + 6]  # [32, 6, 18, 18, 18]
            xqc = np.empty((128, 11664), ml_dtypes.bfloat16)
            for r in range(4):
                xqc[32 * r:32 * r + 32] = \
                    xt[:, :, 4 * r:4 * r + 6].reshape(32, -1)
            in_maps.append({"xq": xqc, "wq": wq, "biasq": bq})
    return in_maps


def run_spmd(x, weight, bias, trace=False, trace_cores=None, tmpdir=None):
    """Returns (output ndarray, BassKernelResults)."""
    from concourse.bass_utils import run_bass_kernel_spmd
    nc = _build()
    in_maps = _prep_inputs(x, weight, bias)
    res = run_bass_kernel_spmd(nc, in_maps, core_ids=list(range(N_CORES)),
                               trace=trace, trace_cores=trace_cores,
                               tmpdir=tmpdir)
    out = np.empty((2, 64, 16, 16, 16, 16), np.float32)
    for c in range(N_CORES):
        b, tq = c // 4, c % 4
        out[b, :, 4 * tq:4 * tq + 4] = \
            res.results[c]["out"].reshape(64, 4, 16, 16, 16)
    return out, res


def kernel(x, weight, bias):
    out, _ = run_spmd(x, weight, bias)
    return out


# revision 4
# speedup vs baseline: 1.5659x; 1.2056x over previous
"""Conv4d (B=2, Ci=32, Co=64, 16^4 spatial, k=3^4, stride 1, pad 1) on 8
Trainium2 NeuronCores.

Sharding: 8 cores = batch(2) x T-quarters(4). Each core computes
out[64co, 4t, 16d, 16h, 16w] for its (b, t-quarter).

Per-core: SBUF x tile [128, 6t*6d*324] bf16 where partition group
r in {0..3} holds ci=32 channels of the padded input restricted to the
D-halo window [4r, 4r+6), planes flattened as 18x18=324.

Full 128x128 PE utilization: 4 row groups (D-quarters, K=32ci) x
2 column groups (even/odd taps, M=64co) = 8 concurrent 32x64 subarray
matmuls. Loop: batch over `to` (4 output t-frames); per batch the 8
PSUM banks hold [128, 512] accumulators for (dp in 2) x (r in 4)
regions; partitions 0-63 accumulate even taps (col group 0),
partitions 64-127 odd taps (col group 1). 81 taps = 41 pair-steps.
Epilogue: DVE adds the two halves, ACT adds bias, DMA out (sync queue).
"""
import sys

sys.path.insert(0, "/opt/trn_rl_repo")
import numpy as np
import ml_dtypes

N_CORES = 8
TAPS = [(kt, kd, kh, kw) for kt in range(3) for kd in range(3)
        for kh in range(3) for kw in range(3)]

_NC = None


def _build():
    global _NC
    if _NC is not None:
        return _NC
    import concourse.bacc as bacc
    import concourse.tile as tile
    from concourse import mybir

    f32 = mybir.dt.float32
    bf16 = mybir.dt.bfloat16

    nc = bacc.Bacc("TRN2", debug=False, target_bir_lowering=False,
                   num_devices=N_CORES)
    xq = nc.dram_tensor("xq", [128, 6 * 6 * 324], bf16, kind="ExternalInput")
    wq = nc.dram_tensor("wq", [32, 81 * 64], bf16, kind="ExternalInput")
    bq = nc.dram_tensor("biasq", [64, 1], f32, kind="ExternalInput")
    out = nc.dram_tensor("out", [64, 16384], f32, kind="ExternalOutput")

    with tile.TileContext(nc) as tc:
        with tc.tile_pool(name="xp", bufs=1) as xp, \
             tc.tile_pool(name="wp", bufs=1) as wp, \
             tc.tile_pool(name="op", bufs=8) as op_, \
             tc.tile_pool(name="pp", bufs=8, space="PSUM") as pp:
            wtile = wp.tile([128, 5184], bf16)
            xtile = xp.tile([128, 11664], bf16)
            btile = wp.tile([64, 1], f32)
            # DMA order tuned so the first matmuls (to=0, kt=0 taps) wait
            # only on w third 0 + x chunk t=0.
            for r in range(4):
                nc.gpsimd.dma_start(wtile[32 * r:32 * r + 32, 0:1728],
                                    wq.ap()[:, 0:1728])
            nc.gpsimd.dma_start(xtile[:, 0:1944], xq.ap()[:, 0:1944])
            nc.gpsimd.dma_start(btile[:], bq.ap()[:])
            for tf in (1, 2):
                nc.gpsimd.dma_start(xtile[:, tf * 1944:(tf + 1) * 1944],
                                    xq.ap()[:, tf * 1944:(tf + 1) * 1944])
            for r in range(4):
                nc.gpsimd.dma_start(wtile[32 * r:32 * r + 32, 1728:3456],
                                    wq.ap()[:, 1728:3456])
            nc.gpsimd.dma_start(xtile[:, 3 * 1944:4 * 1944],
                                xq.ap()[:, 3 * 1944:4 * 1944])
            for r in range(4):
                nc.gpsimd.dma_start(wtile[32 * r:32 * r + 32, 3456:5184],
                                    wq.ap()[:, 3456:5184])
            for tf in (4, 5):
                nc.gpsimd.dma_start(xtile[:, tf * 1944:(tf + 1) * 1944],
                                    xq.ap()[:, tf * 1944:(tf + 1) * 1944])

            xv = xtile.rearrange("p (t d h w) -> p t d h w",
                                 t=6, d=6, h=18, w=18)

            for to in range(4):
                ps = [[pp.tile([128, 512], f32, tag="ps",
                               name=f"ps_{to}_{dp}_{r}") for r in range(4)]
                      for dp in range(2)]
                for step in range(41):
                    for c in range(2):
                        ti = 2 * step + c
                        if ti > 80:
                            continue
                        kt, kd, kh, kw = TAPS[ti]
                        st = step == 0
                        sp = ti >= 79
                        for dp in range(2):
                            for r in range(4):
                                rhs = xv[32 * r:32 * r + 32, to + kt,
                                         2 * dp + kd: 2 * dp + kd + 2,
                                         kh:kh + 16, kw:kw + 16]
                                lhsT = wtile[32 * r:32 * r + 32,
                                             ti * 64:(ti + 1) * 64]
                                nc.tensor.matmul(
                                    ps[dp][r][64 * c:64 * c + 64, :],
                                    lhsT, rhs, start=st, stop=sp,
                                    tile_position=(32 * r, 64 * c))
                for dp in range(2):
                    for r in range(4):
                        ob = op_.tile([64, 512], f32, tag="ob",
                                      name=f"ob_{to}_{dp}_{r}")
                        oa = op_.tile([64, 512], f32, tag="oa",
                                      name=f"oa_{to}_{dp}_{r}")
                        nc.scalar.activation(
                            ob[:], ps[dp][r][64:128, :],
                            mybir.ActivationFunctionType.Identity,
                            bias=btile[:, 0:1])
                        nc.vector.tensor_tensor(oa[:], ps[dp][r][0:64, :],
                                                ob[:], mybir.AluOpType.add)
                        off = to * 4096 + (4 * r + 2 * dp) * 256
                        nc.sync.dma_start(out.ap()[:, off:off + 512], oa[:])
    nc.compile()
    _NC = nc
    return nc


def _prep_inputs(x, weight, bias):
    x = np.asarray(x, dtype=np.float32)
    weight = np.asarray(weight, dtype=np.float32)
    bias = np.asarray(bias, dtype=np.float32)

    w9 = weight.reshape(64, 32, 81).transpose(2, 1, 0)  # [tap, ci, co]
    warr = np.ascontiguousarray(w9.transpose(1, 0, 2)).reshape(32, 81 * 64)
    wq = warr.astype(ml_dtypes.bfloat16)
    bq = bias.reshape(64, 1).astype(np.float32)

    in_maps = []
    for b in range(2):
        xpad = np.pad(x[b], ((0, 0), (1, 1), (1, 1), (1, 1), (1, 1)))
        for tq in range(4):
            xt = xpad[:, 4 * tq:4 * tq + 6]  # [32, 6, 18, 18, 18]
            xqc = np.empty((128, 11664), ml_dtypes.bfloat16)
            for r in range(4):
                xqc[32 * r:32 * r + 32] = \
                    xt[:, :, 4 * r:4 * r + 6].reshape(32, -1)
            in_maps.append({"xq": xqc, "wq": wq, "biasq": bq})
    return in_maps


def run_spmd(x, weight, bias, trace=False, trace_cores=None, tmpdir=None):
    """Returns (output ndarray, BassKernelResults)."""
    from concourse.bass_utils import run_bass_kernel_spmd
    nc = _build()
    in_maps = _prep_inputs(x, weight, bias)
    res = run_bass_kernel_spmd(nc, in_maps, core_ids=list(range(N_CORES)),
                               trace=trace, trace_cores=trace_cores,
                               tmpdir=tmpdir)
    out = np.empty((2, 64, 16, 16, 16, 16), np.float32)
    for c in range(N_CORES):
        b, tq = c // 4, c % 4
        out[b, :, 4 * tq:4 * tq + 4] = \
            res.results[c]["out"].reshape(64, 4, 16, 16, 16)
    return out, res


def kernel(x, weight, bias):
    out, _ = run_spmd(x, weight, bias)
    return out


# revision 5
# speedup vs baseline: 2.0832x; 1.3304x over previous
"""Conv4d (B=2, Ci=32, Co=64, 16^4 spatial, k=3^4, stride 1, pad 1) on 8
Trainium2 NeuronCores.

Sharding: 8 cores = batch(2) x T-quarters(4). Each core computes
out[64co, 4t, 16d, 16h, 16w] for its (b, t-quarter).

The 81 taps are covered by three passes sized to keep the tensor engine
instruction count low (the issue rate ~31ns/inst is the binding
constraint at K=32 granularity):
  A: (kt,kd) in {(0,0),(0,1),(0,2),(1,0)} packed into K=128 (partition
     group g holds x shifted by combo g), M=64, one matmul per (kh,kw).
  B: (kt,kd) in {(1,2),(2,0),(2,1),(2,2)} likewise on a second layout.
  C: (kt,kd)=(1,1) as K=32 matmuls on the quadrant layout (partition
     group r = D-quarter), 4 row groups concurrent.
Each pass splits its 9 (kh,kw) taps across the two 64-wide PE column
groups (tile_position col 0 / 64), accumulating even taps into PSUM
partitions 0-63 and odd taps into 64-127 of one [128,512] bank per
(to, dp) output unit; 8 units (one `to` batch) live at once = 8 banks.
Epilogue: ACT adds bias to the odd half, DVE adds the halves, DMA out.
"""
import sys

sys.path.insert(0, "/opt/trn_rl_repo")
import numpy as np
import ml_dtypes

N_CORES = 8
KHW = [(kh, kw) for kh in range(3) for kw in range(3)]
A_COMBOS = [(0, 0), (0, 1), (0, 2), (1, 0)]
B_COMBOS = [(1, 2), (2, 0), (2, 1), (2, 2)]

_NC = None


def _build():
    global _NC
    if _NC is not None:
        return _NC
    import concourse.bacc as bacc
    import concourse.tile as tile
    from concourse import mybir

    f32 = mybir.dt.float32
    bf16 = mybir.dt.bfloat16

    nc = bacc.Bacc("TRN2", debug=False, target_bir_lowering=False,
                   num_devices=N_CORES)
    xqa = nc.dram_tensor("xqa", [128, 20736], bf16, kind="ExternalInput")
    xqb = nc.dram_tensor("xqb", [128, 20736], bf16, kind="ExternalInput")
    xqc = nc.dram_tensor("xqc", [128, 11664], bf16, kind="ExternalInput")
    wa = nc.dram_tensor("wa", [128, 576], bf16, kind="ExternalInput")
    wb = nc.dram_tensor("wb", [128, 576], bf16, kind="ExternalInput")
    wc = nc.dram_tensor("wc", [128, 576], bf16, kind="ExternalInput")
    bq = nc.dram_tensor("biasq", [64, 1], f32, kind="ExternalInput")
    out = nc.dram_tensor("out", [64, 16384], f32, kind="ExternalOutput")

    with tile.TileContext(nc) as tc:
        with tc.tile_pool(name="xp", bufs=1) as xp, \
             tc.tile_pool(name="wp", bufs=1) as wp, \
             tc.tile_pool(name="op", bufs=8) as op_, \
             tc.tile_pool(name="pp", bufs=8, space="PSUM") as pp:
            wat = wp.tile([128, 576], bf16)
            wbt = wp.tile([128, 576], bf16)
            wct = wp.tile([128, 576], bf16)
            btile = wp.tile([64, 1], f32)
            xat = xp.tile([128, 20736], bf16)
            xbt = xp.tile([128, 20736], bf16)
            xct = xp.tile([128, 11664], bf16)

            # DMA order: batch to needs A/B chunk t0=to and quad chunk
            # t=to+1 (C uses kt=1). 5184 cols per A/B t0-chunk.
            nc.gpsimd.dma_start(wat[:], wa.ap()[:])
            nc.gpsimd.dma_start(wbt[:], wb.ap()[:])
            nc.gpsimd.dma_start(wct[:], wc.ap()[:])
            nc.gpsimd.dma_start(btile[:], bq.ap()[:])
            for t0 in range(4):
                nc.gpsimd.dma_start(xat[:, t0 * 5184:(t0 + 1) * 5184],
                                    xqa.ap()[:, t0 * 5184:(t0 + 1) * 5184])
                nc.gpsimd.dma_start(xbt[:, t0 * 5184:(t0 + 1) * 5184],
                                    xqb.ap()[:, t0 * 5184:(t0 + 1) * 5184])
                nc.gpsimd.dma_start(
                    xct[:, (t0 + 1) * 1944:(t0 + 2) * 1944],
                    xqc.ap()[:, (t0 + 1) * 1944:(t0 + 2) * 1944])

            xav = xat.rearrange("p (t d h w) -> p t d h w",
                                t=4, d=16, h=18, w=18)
            xbv = xbt.rearrange("p (t d h w) -> p t d h w",
                                t=4, d=16, h=18, w=18)
            xcv = xct.rearrange("p (t d h w) -> p t d h w",
                                t=6, d=6, h=18, w=18)

            # chain bookkeeping: per (dp, c) chain position for start/stop
            for to in range(4):
                ps = [pp.tile([128, 512], f32, tag="ps",
                              name=f"ps_{to}_{dp}") for dp in range(8)]
                nch = [[0, 0] for _ in range(8)]   # issued count per (dp,c)
                tot = [[0, 0] for _ in range(8)]
                for pi in range(3):
                    for j in range(9):
                        c = (j + (1 if pi == 1 else 0)) % 2
                        for dp in range(8):
                            tot[dp][c] += 1
                for pi, (wt, xv) in enumerate(((wat, xav), (wbt, xbv),
                                              (wct, xcv))):
                    for j, (kh, kw) in enumerate(KHW):
                        c = (j + (1 if pi == 1 else 0)) % 2
                        for dp in range(8):
                            nch[dp][c] += 1
                            st = nch[dp][c] == 1
                            sp = nch[dp][c] == tot[dp][c]
                            o_ap = ps[dp][64 * c:64 * c + 64, :]
                            if pi < 2:
                                rhs = xv[:, to, 2 * dp:2 * dp + 2,
                                         kh:kh + 16, kw:kw + 16]
                                lhsT = wt[:, j * 64:(j + 1) * 64]
                                nc.tensor.matmul(o_ap, lhsT, rhs,
                                                 start=st, stop=sp,
                                                 tile_position=(0, 64 * c))
                            else:
                                r = dp // 2
                                ld = 2 * (dp % 2) + 1
                                rhs = xv[32 * r:32 * r + 32, to + 1,
                                         ld:ld + 2, kh:kh + 16, kw:kw + 16]
                                lhsT = wt[32 * r:32 * r + 32,
                                          j * 64:(j + 1) * 64]
                                nc.tensor.matmul(o_ap, lhsT, rhs,
                                                 start=st, stop=sp,
                                                 tile_position=(32 * r,
                                                                64 * c))
                for dp in range(8):
                    ob = op_.tile([64, 512], f32, tag="ob",
                                  name=f"ob_{to}_{dp}")
                    oa = op_.tile([64, 512], f32, tag="oa",
                                  name=f"oa_{to}_{dp}")
                    nc.scalar.activation(
                        ob[:], ps[dp][64:128, :],
                        mybir.ActivationFunctionType.Identity,
                        bias=btile[:, 0:1])
                    nc.vector.tensor_tensor(oa[:], ps[dp][0:64, :],
                                            ob[:], mybir.AluOpType.add)
                    off = to * 4096 + dp * 512
                    nc.sync.dma_start(out.ap()[:, off:off + 512], oa[:])
    nc.compile()
    _NC = nc
    return nc


def _prep_inputs(x, weight, bias):
    x = np.asarray(x, dtype=np.float32)
    weight = np.asarray(weight, dtype=np.float32)
    bias = np.asarray(bias, dtype=np.float32)

    def wpack(kt, kd):
        # [32ci, 9khw * 64co]
        return np.ascontiguousarray(
            weight[:, :, kt, kd].reshape(64, 32, 9).transpose(1, 2, 0)
        ).reshape(32, 576)

    wa = np.concatenate([wpack(kt, kd) for kt, kd in A_COMBOS], axis=0)
    wb = np.concatenate([wpack(kt, kd) for kt, kd in B_COMBOS], axis=0)
    wc = np.concatenate([wpack(1, 1)] * 4, axis=0)
    wa = wa.astype(ml_dtypes.bfloat16)
    wb = wb.astype(ml_dtypes.bfloat16)
    wc = wc.astype(ml_dtypes.bfloat16)
    bq = bias.reshape(64, 1).astype(np.float32)

    in_maps = []
    for b in range(2):
        xpad = np.pad(x[b], ((0, 0), (1, 1), (1, 1), (1, 1), (1, 1)))
        for tq in range(4):
            xt = xpad[:, 4 * tq:4 * tq + 6]  # [32, 6t, 18d, 18, 18]
            xa = np.empty((128, 20736), ml_dtypes.bfloat16)
            xb = np.empty((128, 20736), ml_dtypes.bfloat16)
            for g, (kt, kd) in enumerate(A_COMBOS):
                xa[32 * g:32 * g + 32] = \
                    xt[:, kt:kt + 4, kd:kd + 16].reshape(32, -1)
            for g, (kt, kd) in enumerate(B_COMBOS):
                xb[32 * g:32 * g + 32] = \
                    xt[:, kt:kt + 4, kd:kd + 16].reshape(32, -1)
            xc = np.empty((128, 11664), ml_dtypes.bfloat16)
            for r in range(4):
                xc[32 * r:32 * r + 32] = \
                    xt[:, :, 4 * r:4 * r + 6].reshape(32, -1)
            in_maps.append({"xqa": xa, "xqb": xb, "xqc": xc,
                            "wa": wa, "wb": wb, "wc": wc, "biasq": bq})
    return in_maps


def run_spmd(x, weight, bias, trace=False, trace_cores=None, tmpdir=None):
    """Returns (output ndarray, BassKernelResults)."""
    from concourse.bass_utils import run_bass_kernel_spmd
    nc = _build()
    in_maps = _prep_inputs(x, weight, bias)
    res = run_bass_kernel_spmd(nc, in_maps, core_ids=list(range(N_CORES)),
                               trace=trace, trace_cores=trace_cores,
                               tmpdir=tmpdir)
    out = np.empty((2, 64, 16, 16, 16, 16), np.float32)
    for c in range(N_CORES):
        b, tq = c // 4, c % 4
        out[b, :, 4 * tq:4 * tq + 4] = \
            res.results[c]["out"].reshape(64, 4, 16, 16, 16)
    return out, res


def kernel(x, weight, bias):
    out, _ = run_spmd(x, weight, bias)
    return out


# revision 6
# speedup vs baseline: 2.1012x; 1.0086x over previous
"""Conv4d (B=2, Ci=32, Co=64, 16^4 spatial, k=3^4, stride 1, pad 1) on 8
Trainium2 NeuronCores.

Sharding: 8 cores = batch(2) x T-quarters(4). Each core computes
out[64co, 4t, 16d, 16h, 16w] for its (b, t-quarter).

The 81 taps are covered by three passes sized to keep the tensor engine
instruction count low (the ~31ns/inst issue rate binds at K=32):
  A: (kt,kd) in {(0,0),(0,1),(0,2),(1,0)} packed into K=128 (partition
     group g holds x shifted by combo g), M=64, one matmul per (kh,kw).
  B: (kt,kd) in {(1,2),(2,0),(2,1),(2,2)} likewise on a second layout.
  C: (kt,kd)=(1,1) as K=32 matmuls on a cropped quadrant layout
     (partition group r = D-quarter), 4 row groups concurrent, issued
     in two unit-waves so epilogues stagger.
Each pass splits its 9 (kh,kw) taps across the two 64-wide PE column
groups (tile_position col 0 / 64), accumulating even taps into PSUM
partitions 0-63 and odd taps into 64-127 of one [128,512] bank per
(to, dp) output unit; 8 units (one `to` batch) live at once = 8 banks.
Epilogue: ACT adds bias to the odd half, DVE adds the halves, DMA out.
"""
import sys

sys.path.insert(0, "/opt/trn_rl_repo")
import numpy as np
import ml_dtypes

N_CORES = 8
KHW = [(kh, kw) for kh in range(3) for kw in range(3)]
A_COMBOS = [(0, 0), (0, 1), (0, 2), (1, 0)]
B_COMBOS = [(1, 2), (2, 0), (2, 1), (2, 2)]

_NC = None


def _build():
    global _NC
    if _NC is not None:
        return _NC
    import concourse.bacc as bacc
    import concourse.tile as tile
    from concourse import mybir

    f32 = mybir.dt.float32
    bf16 = mybir.dt.bfloat16

    nc = bacc.Bacc("TRN2", debug=False, target_bir_lowering=False,
                   num_devices=N_CORES)
    xqa = nc.dram_tensor("xqa", [128, 20736], bf16, kind="ExternalInput")
    xqb = nc.dram_tensor("xqb", [128, 20736], bf16, kind="ExternalInput")
    xqc = nc.dram_tensor("xqc", [128, 5184], bf16, kind="ExternalInput")
    wa = nc.dram_tensor("wa", [128, 576], bf16, kind="ExternalInput")
    wb = nc.dram_tensor("wb", [128, 576], bf16, kind="ExternalInput")
    wc = nc.dram_tensor("wc", [128, 576], bf16, kind="ExternalInput")
    bq = nc.dram_tensor("biasq", [64, 1], f32, kind="ExternalInput")
    out = nc.dram_tensor("out", [64, 16384], f32, kind="ExternalOutput")

    with tile.TileContext(nc) as tc:
        with tc.tile_pool(name="xp", bufs=1) as xp, \
             tc.tile_pool(name="wp", bufs=1) as wp, \
             tc.tile_pool(name="op", bufs=8) as op_, \
             tc.tile_pool(name="pp", bufs=8, space="PSUM") as pp:
            wat = wp.tile([128, 576], bf16)
            wbt = wp.tile([128, 576], bf16)
            wct = wp.tile([128, 576], bf16)
            btile = wp.tile([64, 1], f32)
            xat = xp.tile([128, 20736], bf16)
            xbt = xp.tile([128, 20736], bf16)
            xct = xp.tile([128, 5184], bf16)

            # Issue order == arrival order (one FIFO input queue feeding
            # all 16 DMA engines). First matmuls need wa + A[t0=0,d 0..3]
            # only, so those go first, d-chunked.
            nc.gpsimd.dma_start(wat[:], wa.ap()[:])
            nc.gpsimd.dma_start(wbt[:], wb.ap()[:])
            for q in range(4):
                nc.gpsimd.dma_start(xat[:, q * 1296:(q + 1) * 1296],
                                    xqa.ap()[:, q * 1296:(q + 1) * 1296])
            for q in range(2):
                nc.gpsimd.dma_start(
                    xbt[:, q * 2592:(q + 1) * 2592],
                    xqb.ap()[:, q * 2592:(q + 1) * 2592])
            nc.gpsimd.dma_start(wct[:], wc.ap()[:])
            nc.gpsimd.dma_start(btile[:], bq.ap()[:])
            nc.gpsimd.dma_start(xct[:, 0:1296], xqc.ap()[:, 0:1296])
            for t0 in range(1, 4):
                nc.gpsimd.dma_start(xat[:, t0 * 5184:(t0 + 1) * 5184],
                                    xqa.ap()[:, t0 * 5184:(t0 + 1) * 5184])
                nc.gpsimd.dma_start(xbt[:, t0 * 5184:(t0 + 1) * 5184],
                                    xqb.ap()[:, t0 * 5184:(t0 + 1) * 5184])
                nc.gpsimd.dma_start(
                    xct[:, t0 * 1296:(t0 + 1) * 1296],
                    xqc.ap()[:, t0 * 1296:(t0 + 1) * 1296])

            xav = xat.rearrange("p (t d h w) -> p t d h w",
                                t=4, d=16, h=18, w=18)
            xbv = xbt.rearrange("p (t d h w) -> p t d h w",
                                t=4, d=16, h=18, w=18)
            xcv = xct.rearrange("p (t d h w) -> p t d h w",
                                t=4, d=4, h=18, w=18)

            for to in range(4):
                ps = [pp.tile([128, 512], f32, tag="ps",
                              name=f"ps_{to}_{dp}") for dp in range(8)]
                nch = [[0, 0] for _ in range(8)]
                tot = [[0, 0] for _ in range(8)]
                for pi in range(3):
                    for j in range(9):
                        c = (j + (1 if pi == 1 else 0)) % 2
                        for dp in range(8):
                            tot[dp][c] += 1

                def mm_ab(xv, wt, j, kh, kw, c, dp):
                    nch[dp][c] += 1
                    nc.tensor.matmul(
                        ps[dp][64 * c:64 * c + 64, :],
                        wt[:, j * 64:(j + 1) * 64],
                        xv[:, to, 2 * dp:2 * dp + 2,
                           kh:kh + 16, kw:kw + 16],
                        start=nch[dp][c] == 1,
                        stop=nch[dp][c] == tot[dp][c],
                        tile_position=(0, 64 * c))

                for pi, (wt, xv) in enumerate(((wat, xav), (wbt, xbv))):
                    for j, (kh, kw) in enumerate(KHW):
                        c = (j + pi) % 2
                        for dp in range(8):
                            mm_ab(xv, wt, j, kh, kw, c, dp)
                # pass C: two waves of units spread over the 4 row groups
                for dp in (0, 2, 4, 6, 1, 3, 5, 7):
                    r = dp // 2
                    ld = 2 * (dp % 2)
                    for j, (kh, kw) in enumerate(KHW):
                        c = j % 2
                        nch[dp][c] += 1
                        nc.tensor.matmul(
                            ps[dp][64 * c:64 * c + 64, :],
                            wct[32 * r:32 * r + 32, j * 64:(j + 1) * 64],
                            xcv[32 * r:32 * r + 32, to, ld:ld + 2,
                                kh:kh + 16, kw:kw + 16],
                            start=nch[dp][c] == 1,
                            stop=nch[dp][c] == tot[dp][c],
                            tile_position=(32 * r, 64 * c))
                for dp in (0, 2, 4, 6, 1, 3, 5, 7):
                    ob = op_.tile([64, 512], f32, tag="ob",
                                  name=f"ob_{to}_{dp}")
                    oa = op_.tile([64, 512], f32, tag="oa",
                                  name=f"oa_{to}_{dp}")
                    nc.scalar.activation(
                        ob[:], ps[dp][64:128, :],
                        mybir.ActivationFunctionType.Identity,
                        bias=btile[:, 0:1])
                    nc.vector.tensor_tensor(oa[:], ps[dp][0:64, :],
                                            ob[:], mybir.AluOpType.add)
                    off = to * 4096 + dp * 512
                    nc.sync.dma_start(out.ap()[:, off:off + 512], oa[:])
    nc.compile()
    _NC = nc
    return nc


def _prep_inputs(x, weight, bias):
    x = np.asarray(x, dtype=np.float32)
    weight = np.asarray(weight, dtype=np.float32)
    bias = np.asarray(bias, dtype=np.float32)

    def wpack(kt, kd):
        # [32ci, 9khw * 64co]
        return np.ascontiguousarray(
            weight[:, :, kt, kd].reshape(64, 32, 9).transpose(1, 2, 0)
        ).reshape(32, 576)

    wa = np.concatenate([wpack(kt, kd) for kt, kd in A_COMBOS], axis=0)
    wb = np.concatenate([wpack(kt, kd) for kt, kd in B_COMBOS], axis=0)
    wc = np.concatenate([wpack(1, 1)] * 4, axis=0)
    wa = wa.astype(ml_dtypes.bfloat16)
    wb = wb.astype(ml_dtypes.bfloat16)
    wc = wc.astype(ml_dtypes.bfloat16)
    bq = bias.reshape(64, 1).astype(np.float32)

    in_maps = []
    for b in range(2):
        xpad = np.pad(x[b], ((0, 0), (1, 1), (1, 1), (1, 1), (1, 1)))
        for tq in range(4):
            xt = xpad[:, 4 * tq:4 * tq + 6]  # [32, 6t, 18d, 18, 18]
            xa = np.empty((128, 20736), ml_dtypes.bfloat16)
            xb = np.empty((128, 20736), ml_dtypes.bfloat16)
            for g, (kt, kd) in enumerate(A_COMBOS):
                xa[32 * g:32 * g + 32] = \
                    xt[:, kt:kt + 4, kd:kd + 16].reshape(32, -1)
            for g, (kt, kd) in enumerate(B_COMBOS):
                xb[32 * g:32 * g + 32] = \
                    xt[:, kt:kt + 4, kd:kd + 16].reshape(32, -1)
            # cropped quadrant layout for pass C (kt=kd=1):
            # t planes 1..4, per-quarter padded-d planes 4r+1..4r+4
            xc = np.empty((128, 5184), ml_dtypes.bfloat16)
            for r in range(4):
                xc[32 * r:32 * r + 32] = \
                    xt[:, 1:5, 4 * r + 1:4 * r + 5].reshape(32, -1)
            in_maps.append({"xqa": xa, "xqb": xb, "xqc": xc,
                            "wa": wa, "wb": wb, "wc": wc, "biasq": bq})
    return in_maps


def run_spmd(x, weight, bias, trace=False, trace_cores=None, tmpdir=None):
    """Returns (output ndarray, BassKernelResults)."""
    from concourse.bass_utils import run_bass_kernel_spmd
    nc = _build()
    in_maps = _prep_inputs(x, weight, bias)
    res = run_bass_kernel_spmd(nc, in_maps, core_ids=list(range(N_CORES)),
                               trace=trace, trace_cores=trace_cores,
                               tmpdir=tmpdir)
    out = np.empty((2, 64, 16, 16, 16, 16), np.float32)
    for c in range(N_CORES):
        b, tq = c // 4, c % 4
        out[b, :, 4 * tq:4 * tq + 4] = \
            res.results[c]["out"].reshape(64, 4, 16, 16, 16)
    return out, res


def kernel(x, weight, bias):
    out, _ = run_spmd(x, weight, bias)
    return out


# revision 9
# speedup vs baseline: 2.1537x; 1.0250x over previous
"""Conv4d (B=2, Ci=32, Co=64, 16^4 spatial, k=3^4, stride 1, pad 1) on 8
Trainium2 NeuronCores.

Sharding: 8 cores = batch(2) x T-quarters(4). Each core computes
out[64co, 4t, 16d, 16h, 16w] for its (b, t-quarter).

The 81 taps are covered by three passes sized to keep the tensor engine
instruction count low (the ~31ns/inst issue rate binds at K=32):
  A: (kt,kd) in {(0,0),(0,1),(0,2),(1,0)} packed into K=128 (partition
     group g holds x shifted by combo g), M=64, one matmul per (kh,kw).
  B: (kt,kd) in {(1,2),(2,0),(2,1),(2,2)} likewise on a second layout.
  C: (kt,kd)=(1,1) as K=32 matmuls on a cropped quadrant layout
     (partition group r = D-quarter), 4 row groups concurrent, issued
     in two unit-waves so epilogues stagger.
Each pass splits its 9 (kh,kw) taps across the two 64-wide PE column
groups (tile_position col 0 / 64), accumulating even taps into PSUM
partitions 0-63 and odd taps into 64-127 of one [128,512] bank per
(to, dp) output unit; 8 units (one `to` batch) live at once = 8 banks.
Epilogue: ACT adds bias to the odd half, DVE adds the halves, DMA out.
"""
import sys

sys.path.insert(0, "/opt/trn_rl_repo")
import numpy as np
import ml_dtypes

N_CORES = 8
KHW = [(kh, kw) for kh in range(3) for kw in range(3)]
A_COMBOS = [(0, 0), (0, 1), (0, 2), (1, 0)]
B_COMBOS = [(1, 2), (2, 0), (2, 1), (2, 2)]

_NC = None


def _build():
    global _NC
    if _NC is not None:
        return _NC
    import concourse.bacc as bacc
    import concourse.tile as tile
    from concourse import mybir

    f32 = mybir.dt.float32
    bf16 = mybir.dt.bfloat16

    nc = bacc.Bacc("TRN2", debug=False, target_bir_lowering=False,
                   num_devices=N_CORES)
    xqa = nc.dram_tensor("xqa", [128, 20736], bf16, kind="ExternalInput")
    xqb = nc.dram_tensor("xqb", [128, 20736], bf16, kind="ExternalInput")
    xqc = nc.dram_tensor("xqc", [128, 5184], bf16, kind="ExternalInput")
    wa = nc.dram_tensor("wa", [128, 576], bf16, kind="ExternalInput")
    wb = nc.dram_tensor("wb", [128, 576], bf16, kind="ExternalInput")
    wc = nc.dram_tensor("wc", [128, 576], bf16, kind="ExternalInput")
    bq = nc.dram_tensor("biasq", [64, 1], f32, kind="ExternalInput")
    out = nc.dram_tensor("out", [64, 16384], f32, kind="ExternalOutput")

    with tile.TileContext(nc) as tc:
        with tc.tile_pool(name="xp", bufs=1) as xp, \
             tc.tile_pool(name="wp", bufs=1) as wp, \
             tc.tile_pool(name="op", bufs=8) as op_, \
             tc.tile_pool(name="pp", bufs=8, space="PSUM") as pp:
            wat = wp.tile([128, 576], bf16)
            wbt = wp.tile([128, 576], bf16)
            wct = wp.tile([128, 576], bf16)
            btile = wp.tile([64, 1], f32)
            xat = xp.tile([128, 20736], bf16)
            xbt = xp.tile([128, 20736], bf16)
            xct = xp.tile([128, 5184], bf16)

            # Issue order == arrival order (one FIFO input queue feeding
            # all 16 DMA engines). First matmuls need wa + A[t0=0,d 0..3]
            # only, so those go first, d-chunked, on the sync queue in
            # case it starts ahead of gpsimd.
            nc.sync.dma_start(wat[:], wa.ap()[:])
            for q in range(4):
                nc.sync.dma_start(xat[:, q * 1296:(q + 1) * 1296],
                                  xqa.ap()[:, q * 1296:(q + 1) * 1296])
            nc.gpsimd.dma_start(wbt[:], wb.ap()[:])
            for q in range(2):
                nc.gpsimd.dma_start(
                    xbt[:, q * 2592:(q + 1) * 2592],
                    xqb.ap()[:, q * 2592:(q + 1) * 2592])
            nc.gpsimd.dma_start(wct[:], wc.ap()[:])
            nc.gpsimd.dma_start(btile[:], bq.ap()[:])
            nc.gpsimd.dma_start(xct[:, 0:1296], xqc.ap()[:, 0:1296])
            for t0 in range(1, 4):
                nc.gpsimd.dma_start(xat[:, t0 * 5184:(t0 + 1) * 5184],
                                    xqa.ap()[:, t0 * 5184:(t0 + 1) * 5184])
                nc.gpsimd.dma_start(xbt[:, t0 * 5184:(t0 + 1) * 5184],
                                    xqb.ap()[:, t0 * 5184:(t0 + 1) * 5184])
                nc.gpsimd.dma_start(
                    xct[:, t0 * 1296:(t0 + 1) * 1296],
                    xqc.ap()[:, t0 * 1296:(t0 + 1) * 1296])

            xav = xat.rearrange("p (t d h w) -> p t d h w",
                                t=4, d=16, h=18, w=18)
            xbv = xbt.rearrange("p (t d h w) -> p t d h w",
                                t=4, d=16, h=18, w=18)
            xcv = xct.rearrange("p (t d h w) -> p t d h w",
                                t=4, d=4, h=18, w=18)

            for to in range(4):
                ps = [pp.tile([128, 512], f32, tag="ps",
                              name=f"ps_{to}_{dp}") for dp in range(8)]
                nch = [[0, 0] for _ in range(8)]
                tot = [[0, 0] for _ in range(8)]
                for pi in range(3):
                    for j in range(9):
                        c = (j + (1 if pi == 1 else 0)) % 2
                        for dp in range(8):
                            tot[dp][c] += 1

                def mm_ab(pi, j, dp):
                    wt, xv = ((wat, xav), (wbt, xbv))[pi]
                    kh, kw = KHW[j]
                    c = (j + pi) % 2
                    nch[dp][c] += 1
                    nc.tensor.matmul(
                        ps[dp][64 * c:64 * c + 64, :],
                        wt[:, j * 64:(j + 1) * 64],
                        xv[:, to, 2 * dp:2 * dp + 2,
                           kh:kh + 16, kw:kw + 16],
                        start=nch[dp][c] == 1,
                        stop=nch[dp][c] == tot[dp][c],
                        tile_position=(0, 64 * c))

                # (E,O)-paired issue so both column halves stream from the
                # first instruction; (A j8, B j0) bridges the pass change.
                PAIRS = [((0, 0), (0, 1)), ((0, 2), (0, 3)),
                         ((0, 4), (0, 5)), ((0, 6), (0, 7)),
                         ((0, 8), (1, 0)), ((1, 1), (1, 2)),
                         ((1, 3), (1, 4)), ((1, 5), (1, 6)),
                         ((1, 7), (1, 8))]
                DPO = (0, 2, 4, 6, 1, 3, 5, 7)
                for (pa, ja), (pb, jb) in PAIRS:
                    for dp in DPO:
                        mm_ab(pa, ja, dp)
                        mm_ab(pb, jb, dp)
                # pass C: two waves of units spread over the 4 row groups
                for dp in (0, 2, 4, 6, 1, 3, 5, 7):
                    r = dp // 2
                    ld = 2 * (dp % 2)
                    for j, (kh, kw) in enumerate(KHW):
                        c = j % 2
                        nch[dp][c] += 1
                        nc.tensor.matmul(
                            ps[dp][64 * c:64 * c + 64, :],
                            wct[32 * r:32 * r + 32, j * 64:(j + 1) * 64],
                            xcv[32 * r:32 * r + 32, to, ld:ld + 2,
                                kh:kh + 16, kw:kw + 16],
                            start=nch[dp][c] == 1,
                            stop=nch[dp][c] == tot[dp][c],
                            tile_position=(32 * r, 64 * c))
                for dp in (0, 2, 4, 6, 1, 3, 5, 7):
                    ob = op_.tile([64, 512], f32, tag="ob",
                                  name=f"ob_{to}_{dp}")
                    oa = op_.tile([64, 512], f32, tag="oa",
                                  name=f"oa_{to}_{dp}")
                    nc.scalar.activation(
                        ob[:], ps[dp][64:128, :],
                        mybir.ActivationFunctionType.Identity,
                        bias=btile[:, 0:1])
                    nc.vector.tensor_tensor(oa[:], ps[dp][0:64, :],
                                            ob[:], mybir.AluOpType.add)
                    off = to * 4096 + dp * 512
                    dq = nc.sync if dp % 4 < 2 else nc.gpsimd
                    dq.dma_start(out.ap()[:, off:off + 512], oa[:])
    nc.compile()
    _NC = nc
    return nc


def _prep_inputs(x, weight, bias):
    x = np.asarray(x, dtype=np.float32)
    weight = np.asarray(weight, dtype=np.float32)
    bias = np.asarray(bias, dtype=np.float32)

    def wpack(kt, kd):
        # [32ci, 9khw * 64co]
        return np.ascontiguousarray(
            weight[:, :, kt, kd].reshape(64, 32, 9).transpose(1, 2, 0)
        ).reshape(32, 576)

    wa = np.concatenate([wpack(kt, kd) for kt, kd in A_COMBOS], axis=0)
    wb = np.concatenate([wpack(kt, kd) for kt, kd in B_COMBOS], axis=0)
    wc = np.concatenate([wpack(1, 1)] * 4, axis=0)
    wa = wa.astype(ml_dtypes.bfloat16)
    wb = wb.astype(ml_dtypes.bfloat16)
    wc = wc.astype(ml_dtypes.bfloat16)
    bq = bias.reshape(64, 1).astype(np.float32)

    in_maps = []
    for b in range(2):
        xpad = np.pad(x[b], ((0, 0), (1, 1), (1, 1), (1, 1), (1, 1)))
        for tq in range(4):
            xt = xpad[:, 4 * tq:4 * tq + 6]  # [32, 6t, 18d, 18, 18]
            xa = np.empty((128, 20736), ml_dtypes.bfloat16)
            xb = np.empty((128, 20736), ml_dtypes.bfloat16)
            for g, (kt, kd) in enumerate(A_COMBOS):
                xa[32 * g:32 * g + 32] = \
                    xt[:, kt:kt + 4, kd:kd + 16].reshape(32, -1)
            for g, (kt, kd) in enumerate(B_COMBOS):
                xb[32 * g:32 * g + 32] = \
                    xt[:, kt:kt + 4, kd:kd + 16].reshape(32, -1)
            # cropped quadrant layout for pass C (kt=kd=1):
            # t planes 1..4, per-quarter padded-d planes 4r+1..4r+4
            xc = np.empty((128, 5184), ml_dtypes.bfloat16)
            for r in range(4):
                xc[32 * r:32 * r + 32] = \
                    xt[:, 1:5, 4 * r + 1:4 * r + 5].reshape(32, -1)
            in_maps.append({"xqa": xa, "xqb": xb, "xqc": xc,
                            "wa": wa, "wb": wb, "wc": wc, "biasq": bq})
    return in_maps


def run_spmd(x, weight, bias, trace=False, trace_cores=None, tmpdir=None):
    """Returns (output ndarray, BassKernelResults)."""
    from concourse.bass_utils import run_bass_kernel_spmd
    nc = _build()
    in_maps = _prep_inputs(x, weight, bias)
    res = run_bass_kernel_spmd(nc, in_maps, core_ids=list(range(N_CORES)),
                               trace=trace, trace_cores=trace_cores,
                               tmpdir=tmpdir)
    out = np.empty((2, 64, 16, 16, 16, 16), np.float32)
    for c in range(N_CORES):
        b, tq = c // 4, c % 4
        out[b, :, 4 * tq:4 * tq + 4] = \
            res.results[c]["out"].reshape(64, 4, 16, 16, 16)
    return out, res


def kernel(x, weight, bias):
    out, _ = run_spmd(x, weight, bias)
    return out
